# revision 3
# baseline (speedup 1.0000x reference)
"""Cross-modal attention TRN2 kernel.

Problem: B=4, N=2048, IN_DIM=DIM=1024, HEADS=8, D_HEAD=128, scale=DIM**-0.5.
  q = x_a @ W_q.T ; k,v = split(x_b @ W_kv.T) ; per-head softmax(q k^T/32) v ;
  out = merge_heads @ W_out.T + b_out

Sharding over 8 cores: core c -> batch b=c//2, head-half hh=c%2 (4 heads,
512 of DIM).  W_q/W_kv column-sharded, W_out row-sharded (Megatron); each
core emits a partial output projection y_cT = (W_out[:, slice] @ O_half)
of shape [DIM, N]; host sums the two head-half partials per batch, adds
b_out, transposes back.

Device layout: everything transposed ([feature, token]) so all matmuls
contract over the partition dim.  Host feeds x^T and W^T (cheap numpy
prep); device does:
  phase 1: Q^T = WqT.T @ xaT, K^T likewise, V (natural [j, dv])
  phase 2: per (head, 1024-token block): dots^T = K_tile^T.T... i.e.
           s^T[j,i] = sum_d K^T[d,j] Q^T[d,i]; exp on ACT (no max
           subtraction -- |s*scale| < ~1 by construction of the problem
           scale); PV and a ones-row matmul (denominator) accumulate over
           j-tiles in PSUM; normalize with reciprocal broadcast.
  phase 3: y^T = WoT.T @ O^T, DMA PSUM->DRAM.
All matmuls run as float32r (full PE rate at N=512).
"""

import numpy as np

B, N, IN_DIM, DIM, HEADS = 4, 2048, 1024, 1024, 8
D_HEAD = DIM // HEADS          # 128
SCALE = DIM ** -0.5            # 1/32
NCORES = 8
HH = HEADS // 2                # 4 heads per core
DVC = HH * D_HEAD              # 512 dv per core
P = 128
KT = IN_DIM // P               # 8 contraction tiles
NJT = N // P                   # 16 j tiles
NIB = N // 512                 # 4 i-blocks of 512
IB2 = N // 1024                # 2 i-blocks of 1024

_TRACE = False
_TRACE_DIR = None
REPS = 1
LAST_EXEC_NS = None
LAST_RESULTS = None
_nc_cache = []


def _build_nc(reps=1):
    import concourse.tile as tile
    from concourse import bacc, mybir

    f32 = mybir.dt.float32
    f32r = mybir.dt.float32r
    Exp = mybir.ActivationFunctionType.Exp

    nc = bacc.Bacc("TRN2", debug=False, num_devices=NCORES)

    xaT = nc.dram_tensor("xaT", [IN_DIM, N], f32r, kind="ExternalInput").ap()
    xbT = nc.dram_tensor("xbT", [IN_DIM, N], f32r, kind="ExternalInput").ap()
    wqT = nc.dram_tensor("wqT", [IN_DIM, DVC], f32r, kind="ExternalInput").ap()
    wkT = nc.dram_tensor("wkT", [IN_DIM, DVC], f32r, kind="ExternalInput").ap()
    wvT = nc.dram_tensor("wvT", [IN_DIM, DVC], f32r, kind="ExternalInput").ap()
    woT = nc.dram_tensor("woT", [DVC, DIM], f32r, kind="ExternalInput").ap()
    ones_d = nc.dram_tensor("ones", [P, 1], f32r, kind="ExternalInput").ap()
    yT = nc.dram_tensor("yT", [DIM, N], f32, kind="ExternalOutput").ap()

    with tile.TileContext(nc) as tc:
      for _rep in range(reps):
        with tc.tile_pool(name="persist", bufs=1) as persist:
            qT_sb = persist.tile([P, HH, N], f32r)      # [d%128, head, i]
            kT_sb = persist.tile([P, HH, N], f32r)      # [d%128, head, j]
            v_sb = persist.tile([P, NJT, DVC], f32r)    # [j%128, jt, dv]
            oT_ts = [[persist.tile([P, 1024], f32r, tag=f"o{h}_{bb}",
                                   name=f"o{h}_{bb}")
                      for bb in range(IB2)] for h in range(HH)]
            ones_sb = persist.tile([P, 1], f32r)
            nc.sync.dma_start(out=ones_sb, in_=ones_d)

            # ---------------- phase 1: projections ----------------
            BW = 256  # streaming block width (>=256 keeps f32r full rate)
            NB = N // BW
            with tc.tile_pool(name="wpool", bufs=1) as wpool, \
                 tc.tile_pool(name="xblk", bufs=3) as xblk, \
                 tc.tile_pool(name="psum1", bufs=4, space="PSUM") as psum1:
                wq_ts = [wpool.tile([P, DVC], f32r, tag=f"wq{kt}", name=f"wq{kt}")
                         for kt in range(KT)]
                wk_ts = [wpool.tile([P, DVC], f32r, tag=f"wk{kt}", name=f"wk{kt}")
                         for kt in range(KT)]
                wv_ts = [wpool.tile([P, DVC], f32r, tag=f"wv{kt}", name=f"wv{kt}")
                         for kt in range(KT)]

                for ib in range(NB):
                    xa_blk = xblk.tile([P, KT, BW], f32r, tag="xblk")
                    nc.sync.dma_start(
                        out=xa_blk,
                        in_=xaT[:, ib * BW:(ib + 1) * BW]
                        .rearrange("(kt p) i -> p kt i", p=P))
                    if ib == 0:
                        # after the first x block so the first matmul's
                        # operands land earliest in DMA queue order
                        for kt in range(KT):
                            nc.sync.dma_start(
                                out=wq_ts[kt],
                                in_=wqT[kt * P:(kt + 1) * P, :])
                    for dt in range(HH):
                        ps = psum1.tile([P, BW], f32, tag="ps1")
                        for kt in range(KT):
                            nc.tensor.matmul(
                                ps,
                                wq_ts[kt][:, dt * P:(dt + 1) * P],
                                xa_blk[:, kt, :],
                                start=(kt == 0), stop=(kt == KT - 1))
                        nc.vector.tensor_copy(
                            qT_sb[:, dt, ib * BW:(ib + 1) * BW], ps)

                for jb in range(NB):
                    xb_blk = xblk.tile([P, KT, BW], f32r, tag="xblk")
                    nc.sync.dma_start(
                        out=xb_blk,
                        in_=xbT[:, jb * BW:(jb + 1) * BW]
                        .rearrange("(kt p) i -> p kt i", p=P))
                    if jb == 0:
                        for kt in range(KT):
                            nc.sync.dma_start(
                                out=wk_ts[kt],
                                in_=wkT[kt * P:(kt + 1) * P, :])
                            nc.sync.dma_start(
                                out=wv_ts[kt],
                                in_=wvT[kt * P:(kt + 1) * P, :])
                    for dt in range(HH):
                        ps = psum1.tile([P, BW], f32, tag="ps1")
                        for kt in range(KT):
                            nc.tensor.matmul(
                                ps,
                                wk_ts[kt][:, dt * P:(dt + 1) * P],
                                xb_blk[:, kt, :],
                                start=(kt == 0), stop=(kt == KT - 1))
                        nc.vector.tensor_copy(
                            kT_sb[:, dt, jb * BW:(jb + 1) * BW], ps)
                    for j2 in range(BW // P):
                        jt = jb * (BW // P) + j2
                        ps = psum1.tile([P, DVC], f32, tag="psv")
                        for kt in range(KT):
                            nc.tensor.matmul(
                                ps,
                                xb_blk[:, kt, j2 * P:(j2 + 1) * P],
                                wv_ts[kt],
                                start=(kt == 0), stop=(kt == KT - 1))
                        nc.vector.tensor_copy(v_sb[:, jt, :], ps)

            # ---------------- phase 2: attention ----------------
            with tc.tile_pool(name="expp", bufs=8) as expp, \
                 tc.tile_pool(name="bcp", bufs=2) as bcp, \
                 tc.tile_pool(name="rcp", bufs=2) as rcp, \
                 tc.tile_pool(name="dotsp", bufs=2, space="PSUM") as dotsp, \
                 tc.tile_pool(name="avp", bufs=1, space="PSUM") as avp, \
                 tc.tile_pool(name="denp", bufs=1, space="PSUM") as denp:
                LAG = 2   # PV/ones trail dots/exp by 2 j-tiles so the PE
                          # never waits on the ACT exp of the current tile
                for ib in range(IB2):
                    for h in range(HH):
                        i0 = ib * 1024
                        po = avp.tile([P, 1024], f32)
                        pd = denp.tile([1, 1024], f32)
                        ets = {}
                        for jt in range(NJT + LAG):
                            if jt < NJT:
                                ps = dotsp.tile([P, 1024], f32, tag="ps")
                                k_l = kT_sb[:, h, jt * P:(jt + 1) * P]
                                for hf in range(2):
                                    nc.tensor.matmul(
                                        ps[:, hf * 512:(hf + 1) * 512],
                                        k_l,
                                        qT_sb[:, h,
                                              i0 + hf * 512:i0 + (hf + 1) * 512],
                                        start=True, stop=True)
                                et = expp.tile([P, 1024], f32r, tag="exp")
                                nc.scalar.activation(et, ps, Exp, scale=SCALE)
                                ets[jt] = et
                            if jt >= LAG:
                                jd = jt - LAG
                                et = ets.pop(jd)
                                v_l = v_sb[:, jd, h * P:(h + 1) * P]
                                for hf in range(2):
                                    sl = slice(hf * 512, (hf + 1) * 512)
                                    nc.tensor.matmul(
                                        po[:, sl], v_l, et[:, sl],
                                        start=(jd == 0), stop=(jd == NJT - 1))
                                    nc.tensor.matmul(
                                        pd[:, sl], ones_sb,
                                        et[:, sl],
                                        start=(jd == 0), stop=(jd == NJT - 1))
                        # drain the PV accumulator to SBUF right away so the
                        # PSUM bank frees for the next block; normalize there.
                        osl = oT_ts[h][ib]
                        nc.vector.tensor_copy(osl, po)
                        rc = rcp.tile([1, 1024], f32, tag="rc")
                        nc.vector.reciprocal(rc, pd)
                        bc = bcp.tile([P, 1024], f32, tag="bc")
                        nc.gpsimd.partition_broadcast(bc, rc)
                        nc.vector.tensor_mul(osl, osl, bc)

                # ---------------- phase 3: output projection ----------------
                # y-psum tiles share the dots pool slots (tag "ps"), which
                # free as the exp of the final j-tiles completes -- a fresh
                # PSUM pool would wait on the whole attention stack instead.
                with tc.tile_pool(name="wop", bufs=1) as wop, \
                     tc.tile_pool(name="ystage", bufs=4) as ystage:
                    wo_sb = wop.tile([P, HH, DIM], f32r)
                    nc.sync.dma_start(
                        out=wo_sb, in_=woT.rearrange("(dt p) e -> p dt e", p=P))
                    for ib in range(NIB):
                        bb, half = divmod(ib, 2)
                        for e8 in range(DIM // P):
                            ps = dotsp.tile([P, 512], f32, tag="ps")
                            for dt in range(HH):
                                nc.tensor.matmul(
                                    ps,
                                    wo_sb[:, dt, e8 * P:(e8 + 1) * P],
                                    oT_ts[dt][bb][:, half * 512:(half + 1) * 512],
                                    start=(dt == 0), stop=(dt == HH - 1))
                            ys = ystage.tile([P, 512], f32, tag="ys")
                            if ib % 2 == 0:
                                nc.vector.tensor_copy(ys, ps)
                            else:
                                nc.scalar.copy(ys, ps)
                            nc.sync.dma_start(
                                out=yT[e8 * P:(e8 + 1) * P,
                                       ib * 512:(ib + 1) * 512],
                                in_=ys)

    nc.compile()
    return nc


_nc_by_reps = {}


def _get_nc(reps=1):
    if reps not in _nc_by_reps:
        _nc_by_reps[reps] = _build_nc(reps)
    return _nc_by_reps[reps]


def kernel(x_a, x_b, W_q, W_kv, W_out, b_out):
    global LAST_EXEC_NS, LAST_RESULTS
    from concourse import bass_utils

    x_a = np.asarray(x_a, dtype=np.float32)
    x_b = np.asarray(x_b, dtype=np.float32)
    W_q = np.asarray(W_q, dtype=np.float32)
    W_kv = np.asarray(W_kv, dtype=np.float32)
    W_out = np.asarray(W_out, dtype=np.float32)
    b_out = np.asarray(b_out, dtype=np.float32)

    nc = _get_nc(REPS)

    xaT = [np.ascontiguousarray(x_a[b].T) for b in range(B)]
    xbT = [np.ascontiguousarray(x_b[b].T) for b in range(B)]
    in_maps = []
    for c in range(NCORES):
        b, hh = divmod(c, 2)
        hs = hh * DVC
        in_maps.append({
            "xaT": xaT[b],
            "xbT": xbT[b],
            "wqT": np.ascontiguousarray(W_q[hs:hs + DVC].T),
            "wkT": np.ascontiguousarray(W_kv[hs:hs + DVC].T),
            "wvT": np.ascontiguousarray(W_kv[DIM + hs:DIM + hs + DVC].T),
            "woT": np.ascontiguousarray(W_out[:, hs:hs + DVC].T),
            "ones": np.ones((P, 1), dtype=np.float32),
        })

    res = bass_utils.run_bass_kernel_spmd(
        nc, in_maps, core_ids=list(range(NCORES)), trace=_TRACE,
        tmpdir=_TRACE_DIR)
    LAST_EXEC_NS = res.exec_time_ns
    LAST_RESULTS = res

    out = np.empty((B, N, DIM), dtype=np.float32)
    for b in range(B):
        acc = res.results[2 * b]["yT"] + res.results[2 * b + 1]["yT"]
        out[b] = acc.T + b_out
    return out


def _make_in_maps(x_a, x_b, W_q, W_kv, W_out):
    xaT = [np.ascontiguousarray(x_a[b].T) for b in range(B)]
    xbT = [np.ascontiguousarray(x_b[b].T) for b in range(B)]
    in_maps = []
    for c in range(NCORES):
        b, hh = divmod(c, 2)
        hs = hh * DVC
        in_maps.append({
            "xaT": xaT[b],
            "xbT": xbT[b],
            "wqT": np.ascontiguousarray(W_q[hs:hs + DVC].T),
            "wkT": np.ascontiguousarray(W_kv[hs:hs + DVC].T),
            "wvT": np.ascontiguousarray(W_kv[DIM + hs:DIM + hs + DVC].T),
            "woT": np.ascontiguousarray(W_out[:, hs:hs + DVC].T),
            "ones": np.ones((P, 1), dtype=np.float32),
        })
    return in_maps


def bench(inputs, reps_pair=(1, 9), iters=5):
    """Measure on-device time per kernel body via rep-delta wall timing."""
    import time
    from concourse import bass_utils
    ins = {k: np.asarray(v, dtype=np.float32) for k, v in inputs.items()
           if k != "b_out"}
    in_maps = _make_in_maps(ins["x_a"], ins["x_b"], ins["W_q"], ins["W_kv"],
                            ins["W_out"])
    walls = {}
    for reps in reps_pair:
        nc = _get_nc(reps)
        # warm-up (compile+cache)
        bass_utils.run_bass_kernel_spmd(nc, in_maps, core_ids=list(range(NCORES)))
        ts = []
        for _ in range(iters):
            t0 = time.perf_counter()
            bass_utils.run_bass_kernel_spmd(nc, in_maps,
                                            core_ids=list(range(NCORES)))
            ts.append(time.perf_counter() - t0)
        walls[reps] = min(ts)
        print(f"reps={reps}: wall min={walls[reps]*1e3:.2f} ms  all={[f'{t*1e3:.1f}' for t in ts]}")
    r0, r1 = reps_pair
    ns = (walls[r1] - walls[r0]) / (r1 - r0) * 1e9
    print(f"per-body device time: {ns:.0f} ns")
    return ns



# revision 4
# speedup vs baseline: 1.3513x; 1.3513x over previous
"""Cross-modal attention TRN2 kernel (v2, bf16).

Problem: B=4, N=2048, IN_DIM=DIM=1024, HEADS=8, D_HEAD=128, scale=DIM**-0.5.
  q = x_a @ W_q.T ; k,v = split(x_b @ W_kv.T) ; per-head softmax(q k^T/32) v ;
  out = merge_heads @ W_out.T + b_out

Sharding over 8 cores: core c -> batch b=c//2, head-half hh=c%2 (4 heads,
512 of DIM).  W_q/W_kv column-sharded, W_out row-sharded (Megatron); each
core emits a partial output projection y_cT = (W_out[:, slice] @ O_half)
of shape [DIM, N] in bf16; host sums the two head-half partials per batch
in fp32, adds b_out, transposes back.

v2 changes vs v1 (fp32r, 401us):
  - all matmul operands bf16 (same 1 cyc/row PE rate, but half DMA, half
    LDWEIGHTS, FWL enabled, 2x DVE rates).  PSUM accumulation stays fp32.
  - softmax denominator no longer computed with per-j-tile ones-matmuls
    (which cost as much PE time as the PV matmuls).  Instead the exp tiles
    are summed over j-tiles with a bf16 binary add-tree on the Vector
    engine, and one [128,128] all-ones stationary matmul per (ib,h) both
    reduces over the 128 j-partitions and broadcasts the denominator to
    all 128 partitions of a PSUM tile.  reciprocal_approx_fast (DVE custom
    op, ~5x faster than InstReciprocal) gives 1/den at [128,1024] shape,
    so nothing runs at 1-partition serial rates and the per-iteration
    normalize chain is off the PE critical path.
  - phase 2 software-pipelined ACROSS (ib,h) iterations (PE never drains,
    so the HAM clock gate stays at 2.4 GHz), with the phase-3 output
    projection matmuls for i-block 0 interleaved into the PE slack of the
    ACT-bound (exp-bound) attention iterations of i-block 1.
"""

import numpy as np
from collections import deque

B, N, IN_DIM, DIM, HEADS = 4, 2048, 1024, 1024, 8
D_HEAD = DIM // HEADS          # 128
SCALE = DIM ** -0.5            # 1/32
NCORES = 8
HH = HEADS // 2                # 4 heads per core
DVC = HH * D_HEAD              # 512 dv per core
P = 128
KT = IN_DIM // P               # 8 contraction tiles
NJT = N // P                   # 16 j tiles
IB2 = N // 1024                # 2 i-blocks of 1024
LAG = 2                        # PV trails dots/exp by LAG j-tiles

_TRACE = False
_TRACE_DIR = None
REPS = 1
LAST_EXEC_NS = None
LAST_RESULTS = None


def _build_nc(reps=1):
    import concourse.tile as tile
    from concourse import bacc, mybir

    f32 = mybir.dt.float32
    bf16 = mybir.dt.bfloat16
    Exp = mybir.ActivationFunctionType.Exp

    nc = bacc.Bacc("TRN2", debug=False, num_devices=NCORES)

    xaT = nc.dram_tensor("xaT", [IN_DIM, N], bf16, kind="ExternalInput").ap()
    xbT = nc.dram_tensor("xbT", [IN_DIM, N], bf16, kind="ExternalInput").ap()
    wqT = nc.dram_tensor("wqT", [IN_DIM, DVC], bf16, kind="ExternalInput").ap()
    wkT = nc.dram_tensor("wkT", [IN_DIM, DVC], bf16, kind="ExternalInput").ap()
    wvT = nc.dram_tensor("wvT", [IN_DIM, DVC], bf16, kind="ExternalInput").ap()
    woT = nc.dram_tensor("woT", [DVC, DIM], bf16, kind="ExternalInput").ap()
    ones_d = nc.dram_tensor("ones", [P, P], bf16, kind="ExternalInput").ap()
    yT = nc.dram_tensor("yT", [DIM, N], bf16, kind="ExternalOutput").ap()

    with tile.TileContext(nc) as tc:
      for _rep in range(reps):
        with tc.tile_pool(name="persist", bufs=1) as persist:
            qT_sb = persist.tile([P, HH, N], bf16, tag="qT")    # [d%128, h, i]
            kT_sb = persist.tile([P, HH, N], bf16, tag="kT")    # [d%128, h, j]
            v_sb = persist.tile([P, NJT, DVC], bf16, tag="v")   # [j%128, jt, dv]
            o_ts = [[persist.tile([P, 1024], bf16, tag=f"o{h}_{bb}",
                                  name=f"o{h}_{bb}")
                     for bb in range(IB2)] for h in range(HH)]
            ones_sb = persist.tile([P, P], bf16, tag="ones")
            wo_sb = persist.tile([P, HH, DIM], bf16, tag="wo")  # [dv%128, h, e]

            # ---------------- phase 1: projections ----------------
            BW = 512
            NB = N // BW                                        # 4 blocks
            with tc.tile_pool(name="wpool", bufs=1) as wpool, \
                 tc.tile_pool(name="xapool", bufs=2) as xapool, \
                 tc.tile_pool(name="xbpool", bufs=2) as xbpool, \
                 tc.tile_pool(name="psum1", bufs=4, space="PSUM") as psum1:
                wq_sb = wpool.tile([P, KT, DVC], bf16, tag="wq")
                wk_sb = wpool.tile([P, KT, DVC], bf16, tag="wk")
                wv_sb = wpool.tile([P, KT, DVC], bf16, tag="wv")

                def new_xa(blk):
                    t = xapool.tile([P, KT, BW], bf16, tag="xa", name="xa_blk")
                    nc.sync.dma_start(
                        out=t,
                        in_=xaT[:, blk * BW:(blk + 1) * BW]
                        .rearrange("(kt p) i -> p kt i", p=P))
                    return t

                def new_xb(blk):
                    t = xbpool.tile([P, KT, BW], bf16, tag="xb", name="xb_blk")
                    nc.sync.dma_start(
                        out=t,
                        in_=xbT[:, blk * BW:(blk + 1) * BW]
                        .rearrange("(kt p) i -> p kt i", p=P))
                    return t

                # DMA order: xa0, wq, xb0, wk, wv, then wo/ones (phase 2)
                xa_blk = new_xa(0)
                nc.sync.dma_start(
                    out=wq_sb, in_=wqT.rearrange("(kt p) d -> p kt d", p=P))
                xb_blk = new_xb(0)
                nc.sync.dma_start(
                    out=wk_sb, in_=wkT.rearrange("(kt p) d -> p kt d", p=P))
                nc.sync.dma_start(
                    out=wv_sb, in_=wvT.rearrange("(kt p) d -> p kt d", p=P))
                nc.sync.dma_start(out=ones_sb, in_=ones_d)
                nc.sync.dma_start(
                    out=wo_sb, in_=woT.rearrange("(dt p) e -> p dt e", p=P))

                for blk in range(NB):
                    if blk > 0:
                        xa_blk = new_xa(blk)
                        xb_blk = new_xb(blk)
                    # Q block
                    for dt in range(HH):
                        ps = psum1.tile([P, BW], f32, tag="ps1", name="ps1")
                        for kt in range(KT):
                            nc.tensor.matmul(
                                ps, wq_sb[:, kt, dt * P:(dt + 1) * P],
                                xa_blk[:, kt, :],
                                start=(kt == 0), stop=(kt == KT - 1))
                        nc.vector.tensor_copy(
                            qT_sb[:, dt, blk * BW:(blk + 1) * BW], ps)
                    # K block
                    for dt in range(HH):
                        ps = psum1.tile([P, BW], f32, tag="ps1", name="ps1")
                        for kt in range(KT):
                            nc.tensor.matmul(
                                ps, wk_sb[:, kt, dt * P:(dt + 1) * P],
                                xb_blk[:, kt, :],
                                start=(kt == 0), stop=(kt == KT - 1))
                        nc.vector.tensor_copy(
                            kT_sb[:, dt, blk * BW:(blk + 1) * BW], ps)
                    # V block (j-partitioned: stationary = x slice)
                    for j2 in range(BW // P):
                        ps = psum1.tile([P, DVC], f32, tag="ps1", name="psv")
                        for kt in range(KT):
                            nc.tensor.matmul(
                                ps, xb_blk[:, kt, j2 * P:(j2 + 1) * P],
                                wv_sb[:, kt, :],
                                start=(kt == 0), stop=(kt == KT - 1))
                        nc.vector.tensor_copy(
                            v_sb[:, blk * (BW // P) + j2, :], ps)

            # ---------------- phase 2 + 3: attention + out-proj ----------
            with tc.tile_pool(name="expp", bufs=6) as expp, \
                 tc.tile_pool(name="treep", bufs=6) as treep, \
                 tc.tile_pool(name="rcp", bufs=2) as rcp, \
                 tc.tile_pool(name="ysp", bufs=4) as ysp, \
                 tc.tile_pool(name="dotsp", bufs=2, space="PSUM") as dotsp, \
                 tc.tile_pool(name="avp", bufs=1, space="PSUM") as avp, \
                 tc.tile_pool(name="psyp", bufs=1, space="PSUM") as psyp:

                iters = [(ib, h) for ib in range(IB2) for h in range(HH)]
                pv_pend = deque()      # (k, jd, et)
                ph3_queue = deque()    # (ib, e8, hf)
                state = {}             # k -> dict(acc, pd, rc, po)

                def feed_tree(levels, cur):
                    lvl = 0
                    while levels[lvl] is not None:
                        prev = levels[lvl]
                        levels[lvl] = None
                        with nc.allow_low_precision("softmax denom tree bf16"):
                            dst = treep.tile([P, 1024], bf16, tag="tree",
                                             name="tree")
                            nc.vector.tensor_add(dst, prev, cur)
                        cur = dst
                        lvl += 1
                    levels[lvl] = cur

                def emit_pv(kk, jd, et):
                    st = state[kk]
                    _, hh_ = iters[kk]
                    v_l = v_sb[:, jd, hh_ * P:(hh_ + 1) * P]
                    for hf in range(2):
                        sl = slice(hf * 512, (hf + 1) * 512)
                        nc.tensor.matmul(
                            st["po"][:, sl], v_l, et[:, sl],
                            start=(jd == 0), stop=(jd == NJT - 1))
                    if jd == NJT - 1:
                        # drain PV accumulator right away so the single
                        # avp buffer frees for the next iteration
                        ib_, hh2 = iters[kk]
                        nc.vector.tensor_copy(o_ts[hh2][ib_], st["po"])

                def emit_den(kk):
                    st = state[kk]
                    pd = psyp.tile([P, 1024], f32, tag="psy", name="pden")
                    for hf in range(2):
                        sl = slice(hf * 512, (hf + 1) * 512)
                        nc.tensor.matmul(pd[:, sl], ones_sb, st["acc"][:, sl],
                                         start=True, stop=True)
                    st["pd"] = pd

                def emit_recip(kk):
                    st = state[kk]
                    rc = rcp.tile([P, 1024], f32, tag="rc", name="rc")
                    nc.vector.reciprocal_approx_fast(rc, st["pd"])
                    st["rc"] = rc

                def emit_norm(kk):
                    st = state[kk]
                    ib_, hh_ = iters[kk]
                    osl = o_ts[hh_][ib_]
                    with nc.allow_low_precision("softmax normalize bf16"):
                        nc.vector.tensor_mul(osl, osl, st["rc"])

                def emit_ph3_chunk(ib_, e8, hf, copy_eng="v"):
                    pt = psyp.tile([P, 1024], f32, tag="psy", name="py")
                    pz = pt[:, 0:512]
                    for dt in range(HH):
                        nc.tensor.matmul(
                            pz, wo_sb[:, dt, e8 * P:(e8 + 1) * P],
                            o_ts[dt][ib_][:, hf * 512:(hf + 1) * 512],
                            start=(dt == 0), stop=(dt == HH - 1))
                    ys = ysp.tile([P, 512], bf16, tag="ys", name="ys")
                    if copy_eng == "v":
                        nc.vector.tensor_copy(ys, pz)
                    else:
                        nc.scalar.copy(ys, pz)
                    i0 = ib_ * 1024 + hf * 512
                    nc.sync.dma_start(
                        out=yT[e8 * P:(e8 + 1) * P, i0:i0 + 512], in_=ys)

                for k, (ib, h) in enumerate(iters):
                    st = {"po": avp.tile([P, 1024], f32, tag="po", name="po")}
                    state[k] = st
                    levels = [None] * 5
                    i_base = ib * 1024

                    for jt in range(NJT):
                        # deferred post-iteration work for k-1, spread over
                        # early steps so nothing serializes the PE
                        if k > 0:
                            if jt == 2:
                                emit_den(k - 1)
                            elif jt == 3:
                                emit_recip(k - 1)
                            elif jt == 5:
                                emit_norm(k - 1)

                        # dots
                        ps = dotsp.tile([P, 1024], f32, tag="ps", name="ps")
                        k_l = kT_sb[:, h, jt * P:(jt + 1) * P]
                        for hf in range(2):
                            nc.tensor.matmul(
                                ps[:, hf * 512:(hf + 1) * 512], k_l,
                                qT_sb[:, h,
                                      i_base + hf * 512:i_base + (hf + 1) * 512],
                                start=True, stop=True)
                        et = expp.tile([P, 1024], bf16, tag="exp", name="et")
                        nc.scalar.activation(et, ps, Exp, scale=SCALE)
                        feed_tree(levels, et)
                        pv_pend.append((k, jt, et))
                        if len(pv_pend) > LAG:
                            emit_pv(*pv_pend.popleft())

                        # interleave phase-3 chunks of the previous i-block
                        # into the PE slack (ACT-bound steps); jt>=5 keeps
                        # the psy slot free for den/recip of k-1 and (at
                        # k==4) waits for osl(ib0,h3) to be normalized
                        if ph3_queue and jt in (5, 8, 11, 14):
                            emit_ph3_chunk(*ph3_queue.popleft())

                    st["acc"] = levels[4]
                    assert st["acc"] is not None

                    if h == HH - 1:
                        for e8 in range(DIM // P):
                            for hf in range(2):
                                ph3_queue.append((ib, e8, hf))

                # tail: drain pipeline for the last iteration
                while pv_pend:
                    emit_pv(*pv_pend.popleft())
                kl = len(iters) - 1
                emit_den(kl)
                emit_recip(kl)
                emit_norm(kl)
                ci = 0
                while ph3_queue:
                    emit_ph3_chunk(*ph3_queue.popleft(),
                                   copy_eng=("v" if ci % 2 == 0 else "s"))
                    ci += 1

    nc.compile()
    return nc


_nc_by_reps = {}


def _get_nc(reps=1):
    if reps not in _nc_by_reps:
        _nc_by_reps[reps] = _build_nc(reps)
    return _nc_by_reps[reps]


def _make_in_maps(x_a, x_b, W_q, W_kv, W_out):
    import ml_dtypes
    bf = ml_dtypes.bfloat16
    xaT = [np.ascontiguousarray(x_a[b].T).astype(bf) for b in range(B)]
    xbT = [np.ascontiguousarray(x_b[b].T).astype(bf) for b in range(B)]
    ones = np.ones((P, P), dtype=bf)
    in_maps = []
    for c in range(NCORES):
        b, hh = divmod(c, 2)
        hs = hh * DVC
        in_maps.append({
            "xaT": xaT[b],
            "xbT": xbT[b],
            "wqT": np.ascontiguousarray(W_q[hs:hs + DVC].T).astype(bf),
            "wkT": np.ascontiguousarray(W_kv[hs:hs + DVC].T).astype(bf),
            "wvT": np.ascontiguousarray(
                W_kv[DIM + hs:DIM + hs + DVC].T).astype(bf),
            "woT": np.ascontiguousarray(W_out[:, hs:hs + DVC].T).astype(bf),
            "ones": ones,
        })
    return in_maps


def kernel(x_a, x_b, W_q, W_kv, W_out, b_out):
    global LAST_EXEC_NS, LAST_RESULTS
    from concourse import bass_utils

    x_a = np.asarray(x_a, dtype=np.float32)
    x_b = np.asarray(x_b, dtype=np.float32)
    W_q = np.asarray(W_q, dtype=np.float32)
    W_kv = np.asarray(W_kv, dtype=np.float32)
    W_out = np.asarray(W_out, dtype=np.float32)
    b_out = np.asarray(b_out, dtype=np.float32)

    nc = _get_nc(REPS)
    in_maps = _make_in_maps(x_a, x_b, W_q, W_kv, W_out)

    res = bass_utils.run_bass_kernel_spmd(
        nc, in_maps, core_ids=list(range(NCORES)), trace=_TRACE,
        tmpdir=_TRACE_DIR)
    LAST_EXEC_NS = res.exec_time_ns
    LAST_RESULTS = res

    out = np.empty((B, N, DIM), dtype=np.float32)
    for b in range(B):
        acc = (np.asarray(res.results[2 * b]["yT"]).astype(np.float32)
               + np.asarray(res.results[2 * b + 1]["yT"]).astype(np.float32))
        out[b] = acc.T + b_out
    return out


def bench(inputs, reps_pair=(1, 9), iters=5):
    """Measure on-device time per kernel body via rep-delta wall timing."""
    import time
    from concourse import bass_utils
    ins = {k: np.asarray(v, dtype=np.float32) for k, v in inputs.items()
           if k != "b_out"}
    in_maps = _make_in_maps(ins["x_a"], ins["x_b"], ins["W_q"], ins["W_kv"],
                            ins["W_out"])
    walls = {}
    for reps in reps_pair:
        nc = _get_nc(reps)
        bass_utils.run_bass_kernel_spmd(nc, in_maps, core_ids=list(range(NCORES)))
        ts = []
        for _ in range(iters):
            t0 = time.perf_counter()
            bass_utils.run_bass_kernel_spmd(nc, in_maps,
                                            core_ids=list(range(NCORES)))
            ts.append(time.perf_counter() - t0)
        walls[reps] = min(ts)
        print(f"reps={reps}: wall min={walls[reps]*1e3:.2f} ms  "
              f"all={[f'{t*1e3:.1f}' for t in ts]}")
    r0, r1 = reps_pair
    ns = (walls[r1] - walls[r0]) / (r1 - r0) * 1e9
    print(f"per-body device time: {ns:.0f} ns")
    return ns


# revision 9
# speedup vs baseline: 1.6119x; 1.1928x over previous
"""Cross-modal attention TRN2 kernel (v2, bf16).

Problem: B=4, N=2048, IN_DIM=DIM=1024, HEADS=8, D_HEAD=128, scale=DIM**-0.5.
  q = x_a @ W_q.T ; k,v = split(x_b @ W_kv.T) ; per-head softmax(q k^T/32) v ;
  out = merge_heads @ W_out.T + b_out

Sharding over 8 cores: core c -> batch b=c//2, head-half hh=c%2 (4 heads,
512 of DIM).  W_q/W_kv column-sharded, W_out row-sharded (Megatron); each
core emits a partial output projection y_cT = (W_out[:, slice] @ O_half)
of shape [DIM, N] in bf16; host sums the two head-half partials per batch
in fp32, adds b_out, transposes back.

v2 changes vs v1 (fp32r, 401us):
  - all matmul operands bf16 (same 1 cyc/row PE rate, but half DMA, half
    LDWEIGHTS, FWL enabled, 2x DVE rates).  PSUM accumulation stays fp32.
  - softmax denominator no longer computed with per-j-tile ones-matmuls
    (which cost as much PE time as the PV matmuls).  Instead the exp tiles
    are summed over j-tiles with a bf16 binary add-tree on the Vector
    engine, and one [128,128] all-ones stationary matmul per (ib,h) both
    reduces over the 128 j-partitions and broadcasts the denominator to
    all 128 partitions of a PSUM tile.  reciprocal_approx_fast (DVE custom
    op, ~5x faster than InstReciprocal) gives 1/den at [128,1024] shape,
    so nothing runs at 1-partition serial rates and the per-iteration
    normalize chain is off the PE critical path.
  - phase 2 software-pipelined ACROSS (ib,h) iterations (PE never drains,
    so the HAM clock gate stays at 2.4 GHz), with the phase-3 output
    projection matmuls for i-block 0 interleaved into the PE slack of the
    ACT-bound (exp-bound) attention iterations of i-block 1.
"""

import numpy as np
from collections import deque

B, N, IN_DIM, DIM, HEADS = 4, 2048, 1024, 1024, 8
D_HEAD = DIM // HEADS          # 128
SCALE = DIM ** -0.5            # 1/32
NCORES = 8
HH = HEADS // 2                # 4 heads per core
DVC = HH * D_HEAD              # 512 dv per core
P = 128
KT = IN_DIM // P               # 8 contraction tiles
NJT = N // P                   # 16 j tiles
IB2 = N // 1024                # 2 i-blocks of 1024
LAG = 2                        # PV trails dots/exp by LAG j-tiles

_TRACE = False
_TRACE_DIR = None
REPS = 1
LAST_EXEC_NS = None
LAST_RESULTS = None


def _build_nc(reps=1):
    import concourse.tile as tile
    from concourse import bacc, mybir

    f32 = mybir.dt.float32
    bf16 = mybir.dt.bfloat16
    f8 = mybir.dt.float8e4
    DR = mybir.MatmulPerfMode.DoubleRow
    Exp = mybir.ActivationFunctionType.Exp

    nc = bacc.Bacc("TRN2", debug=False, num_devices=NCORES)

    # Q/K projections run in fp8e4 DoubleRow (weights pre-scaled x16 on the
    # host; the x256 on the logits is folded into the exp scale).  V (and
    # everything downstream) stays bf16: a 2-3% fp8 error on v propagates
    # straight into the output, while on q/k it only perturbs the softmax
    # logits by ~0.6%.
    xa8 = nc.dram_tensor("xa8", [IN_DIM, N], f8, kind="ExternalInput").ap()
    xb8 = nc.dram_tensor("xb8", [IN_DIM, N], f8, kind="ExternalInput").ap()
    xbT = nc.dram_tensor("xbT", [IN_DIM, N], bf16, kind="ExternalInput").ap()
    wq8 = nc.dram_tensor("wq8", [IN_DIM, DVC], f8, kind="ExternalInput").ap()
    wk8 = nc.dram_tensor("wk8", [IN_DIM, DVC], f8, kind="ExternalInput").ap()
    wvT = nc.dram_tensor("wvT", [IN_DIM, DVC], bf16, kind="ExternalInput").ap()
    woT = nc.dram_tensor("woT", [DVC, DIM], bf16, kind="ExternalInput").ap()
    ones_d = nc.dram_tensor("ones", [P, P], bf16, kind="ExternalInput").ap()
    yT = nc.dram_tensor("yT", [DIM, N], bf16, kind="ExternalOutput").ap()

    with tile.TileContext(nc) as tc:
      for _rep in range(reps):
        with tc.tile_pool(name="persist", bufs=1) as persist:
            qT_sb = persist.tile([P, HH, N], bf16, tag="qT")    # [d%128, h, i]
            kT_sb = persist.tile([P, HH, N], bf16, tag="kT")    # [d%128, h, j]
            v_sb = persist.tile([P, NJT, DVC], bf16, tag="v")   # [j%128, jt, dv]
            o_ts = [[persist.tile([P, 1024], bf16, tag=f"o{h}_{bb}",
                                  name=f"o{h}_{bb}")
                     for bb in range(IB2)] for h in range(HH)]
            ones_sb = persist.tile([P, P], bf16, tag="ones")
            wo_sb = persist.tile([P, HH, DIM], bf16, tag="wo")  # [dv%128, h, e]

            # ---------------- phase 1: projections ----------------
            BW = 512
            NB = N // BW                                        # 4 blocks
            KTP = KT // 2                                       # fp8 kt pairs
            with tc.tile_pool(name="wpool", bufs=1) as wpool, \
                 tc.tile_pool(name="xapool", bufs=2) as xapool, \
                 tc.tile_pool(name="xb8pool", bufs=2) as xb8pool, \
                 tc.tile_pool(name="xbpool", bufs=2) as xbpool, \
                 tc.tile_pool(name="psum1", bufs=4, space="PSUM") as psum1:
                wq_sb = wpool.tile([P, KTP, 2, DVC], f8, tag="wq")
                wk_sb = wpool.tile([P, KTP, 2, DVC], f8, tag="wk")
                wv_sb = wpool.tile([P, KT, DVC], bf16, tag="wv")

                def new_x8(pool, dram, blk, nm):
                    t = pool.tile([P, KTP, 2, BW], f8, tag=nm, name=nm)
                    nc.sync.dma_start(
                        out=t,
                        in_=dram[:, blk * BW:(blk + 1) * BW]
                        .rearrange("(ktp ko p) i -> p ktp ko i", p=P, ko=2))
                    return t

                def new_xb(blk):
                    t = xbpool.tile([P, KT, BW], bf16, tag="xb", name="xb_blk")
                    nc.sync.dma_start(
                        out=t,
                        in_=xbT[:, blk * BW:(blk + 1) * BW]
                        .rearrange("(kt p) i -> p kt i", p=P))
                    return t

                # DMA order: earliest-needed first
                xa_blk = new_x8(xapool, xa8, 0, "xa")
                nc.sync.dma_start(
                    out=wq_sb,
                    in_=wq8.rearrange("(ktp ko p) d -> p ktp ko d", p=P, ko=2))
                xb8_blk = new_x8(xb8pool, xb8, 0, "x8")
                nc.sync.dma_start(
                    out=wk_sb,
                    in_=wk8.rearrange("(ktp ko p) d -> p ktp ko d", p=P, ko=2))
                xb_blk = new_xb(0)
                nc.sync.dma_start(
                    out=wv_sb, in_=wvT.rearrange("(kt p) d -> p kt d", p=P))
                nc.sync.dma_start(out=ones_sb, in_=ones_d)
                nc.sync.dma_start(
                    out=wo_sb, in_=woT.rearrange("(dt p) e -> p dt e", p=P))

                for blk in range(NB):
                    if blk > 0:
                        xa_blk = new_x8(xapool, xa8, blk, "xa")
                        xb8_blk = new_x8(xb8pool, xb8, blk, "x8")
                        xb_blk = new_xb(blk)
                    # Q block (fp8 DoubleRow over kt pairs)
                    for dt in range(HH):
                        ps = psum1.tile([P, BW], f32, tag="ps1", name="ps1")
                        for kp in range(KTP):
                            nc.tensor.matmul(
                                ps, wq_sb[:, kp, :, dt * P:(dt + 1) * P],
                                xa_blk[:, kp], perf_mode=DR,
                                start=(kp == 0), stop=(kp == KTP - 1))
                        nc.vector.tensor_copy(
                            qT_sb[:, dt, blk * BW:(blk + 1) * BW], ps)
                    # K block (fp8 DoubleRow)
                    for dt in range(HH):
                        ps = psum1.tile([P, BW], f32, tag="ps1", name="ps1")
                        for kp in range(KTP):
                            nc.tensor.matmul(
                                ps, wk_sb[:, kp, :, dt * P:(dt + 1) * P],
                                xb8_blk[:, kp], perf_mode=DR,
                                start=(kp == 0), stop=(kp == KTP - 1))
                        nc.vector.tensor_copy(
                            kT_sb[:, dt, blk * BW:(blk + 1) * BW], ps)
                    # V block (bf16, j-partitioned: stationary = x slice)
                    for j2 in range(BW // P):
                        ps = psum1.tile([P, DVC], f32, tag="ps1", name="psv")
                        for kt in range(KT):
                            nc.tensor.matmul(
                                ps, xb_blk[:, kt, j2 * P:(j2 + 1) * P],
                                wv_sb[:, kt, :],
                                start=(kt == 0), stop=(kt == KT - 1))
                        nc.vector.tensor_copy(
                            v_sb[:, blk * (BW // P) + j2, :], ps)

            # ---------------- phase 2 + 3: attention + out-proj ----------
            with tc.tile_pool(name="expp", bufs=6) as expp, \
                 tc.tile_pool(name="treep", bufs=6) as treep, \
                 tc.tile_pool(name="rcp", bufs=2) as rcp, \
                 tc.tile_pool(name="ysp", bufs=4) as ysp, \
                 tc.tile_pool(name="dotsp", bufs=2, space="PSUM") as dotsp, \
                 tc.tile_pool(name="avp", bufs=1, space="PSUM") as avp, \
                 tc.tile_pool(name="psyp", bufs=1, space="PSUM") as psyp:

                iters = [(ib, h) for ib in range(IB2) for h in range(HH)]
                pv_pend = deque()      # (k, jd, et)
                ph3_queue = deque()    # (ib, e8, hf)
                state = {}             # k -> dict(acc, pd, rc, po)

                def feed_tree(levels, cur):
                    lvl = 0
                    while levels[lvl] is not None:
                        prev = levels[lvl]
                        levels[lvl] = None
                        with nc.allow_low_precision("softmax denom tree bf16"):
                            dst = treep.tile([P, 1024], bf16, tag="tree",
                                             name="tree")
                            nc.vector.tensor_add(dst, prev, cur)
                        cur = dst
                        lvl += 1
                    levels[lvl] = cur

                def emit_pv(kk, jd, et):
                    st = state[kk]
                    _, hh_ = iters[kk]
                    v_l = v_sb[:, jd, hh_ * P:(hh_ + 1) * P]
                    for hf in range(2):
                        sl = slice(hf * 512, (hf + 1) * 512)
                        nc.tensor.matmul(
                            st["po"][:, sl], v_l, et[:, sl],
                            start=(jd == 0), stop=(jd == NJT - 1))
                    if jd == NJT - 1:
                        # drain PV accumulator right away so the single
                        # avp buffer frees for the next iteration
                        ib_, hh2 = iters[kk]
                        nc.vector.tensor_copy(o_ts[hh2][ib_], st["po"])

                def emit_den(kk):
                    st = state[kk]
                    pd = psyp.tile([P, 1024], f32, tag="psy", name="pden")
                    for hf in range(2):
                        sl = slice(hf * 512, (hf + 1) * 512)
                        nc.tensor.matmul(pd[:, sl], ones_sb, st["acc"][:, sl],
                                         start=True, stop=True)
                    st["pd"] = pd

                def emit_recip(kk):
                    st = state[kk]
                    rc = rcp.tile([P, 1024], f32, tag="rc", name="rc")
                    nc.vector.reciprocal_approx_fast(rc, st["pd"])
                    st["rc"] = rc

                def emit_norm(kk):
                    st = state[kk]
                    ib_, hh_ = iters[kk]
                    osl = o_ts[hh_][ib_]
                    with nc.allow_low_precision("softmax normalize bf16"):
                        nc.vector.tensor_mul(osl, osl, st["rc"])

                def emit_ph3_chunk(ib_, e8, hf, copy_eng="v"):
                    pt = psyp.tile([P, 1024], f32, tag="psy", name="py")
                    pz = pt[:, 0:512]
                    for dt in range(HH):
                        nc.tensor.matmul(
                            pz, wo_sb[:, dt, e8 * P:(e8 + 1) * P],
                            o_ts[dt][ib_][:, hf * 512:(hf + 1) * 512],
                            start=(dt == 0), stop=(dt == HH - 1))
                    ys = ysp.tile([P, 512], bf16, tag="ys", name="ys")
                    if copy_eng == "v":
                        nc.vector.tensor_copy(ys, pz)
                    else:
                        nc.scalar.copy(ys, pz)
                    i0 = ib_ * 1024 + hf * 512
                    nc.sync.dma_start(
                        out=yT[e8 * P:(e8 + 1) * P, i0:i0 + 512], in_=ys)

                for k, (ib, h) in enumerate(iters):
                    st = {"po": avp.tile([P, 1024], f32, tag="po", name="po")}
                    state[k] = st
                    levels = [None] * 5
                    i_base = ib * 1024

                    for jt in range(NJT):
                        # deferred post-iteration work for k-1, spread over
                        # early steps so nothing serializes the PE
                        if k > 0:
                            if jt == 2:
                                emit_den(k - 1)
                            elif jt == 3:
                                emit_recip(k - 1)
                            elif jt == 5:
                                emit_norm(k - 1)

                        # dots
                        ps = dotsp.tile([P, 1024], f32, tag="ps", name="ps")
                        k_l = kT_sb[:, h, jt * P:(jt + 1) * P]
                        for hf in range(2):
                            nc.tensor.matmul(
                                ps[:, hf * 512:(hf + 1) * 512], k_l,
                                qT_sb[:, h,
                                      i_base + hf * 512:i_base + (hf + 1) * 512],
                                start=True, stop=True)
                        et = expp.tile([P, 1024], bf16, tag="exp", name="et")
                        # q,k carry a x16 fp8 pre-scale each -> logits x256
                        nc.scalar.activation(et, ps, Exp, scale=SCALE / 256.0)
                        feed_tree(levels, et)
                        pv_pend.append((k, jt, et))
                        if len(pv_pend) > LAG:
                            emit_pv(*pv_pend.popleft())

                        # interleave phase-3 chunks of the previous i-block
                        # into the PE slack (ACT-bound steps); jt>=5 keeps
                        # the psy slot free for den/recip of k-1 and (at
                        # k==4) waits for osl(ib0,h3) to be normalized
                        if ph3_queue and jt in (5, 8, 11, 14):
                            emit_ph3_chunk(*ph3_queue.popleft())

                    st["acc"] = levels[4]
                    assert st["acc"] is not None

                    if h == HH - 1:
                        for e8 in range(DIM // P):
                            for hf in range(2):
                                ph3_queue.append((ib, e8, hf))

                # tail: drain the pipeline for the last iteration.  The ib1
                # out-proj chunks run 2-deep through the (now free) dots
                # PSUM pool, with their dt<3 accumulations pre-running while
                # the final denominator chain completes on DVE, so the PE
                # never idles long enough for HAM to re-throttle the clock.
                while pv_pend:
                    emit_pv(*pv_pend.popleft())
                kl = len(iters) - 1
                tail = list(ph3_queue)
                ph3_queue.clear()
                open_ps = {}
                ci = 0
                for i in range(len(tail) + 2):
                    if i < len(tail):
                        ib_, e8, hf = tail[i]
                        pt = dotsp.tile([P, 1024], f32, tag="ps", name="py2")
                        pz = pt[:, 0:512]
                        for dt in range(HH - 1):
                            nc.tensor.matmul(
                                pz, wo_sb[:, dt, e8 * P:(e8 + 1) * P],
                                o_ts[dt][ib_][:, hf * 512:(hf + 1) * 512],
                                start=(dt == 0), stop=False)
                        open_ps[i] = pt
                    if i == 0:
                        emit_den(kl)
                        emit_recip(kl)
                    elif i == 1:
                        emit_norm(kl)
                    if i >= 2:
                        j = i - 2
                        ib_, e8, hf = tail[j]
                        pt = open_ps.pop(j)
                        pz = pt[:, 0:512]
                        nc.tensor.matmul(
                            pz, wo_sb[:, HH - 1, e8 * P:(e8 + 1) * P],
                            o_ts[HH - 1][ib_][:, hf * 512:(hf + 1) * 512],
                            start=False, stop=True)
                        ys = ysp.tile([P, 512], bf16, tag="ys", name="ys")
                        if ci % 2 == 0:
                            nc.vector.tensor_copy(ys, pz)
                        else:
                            nc.scalar.copy(ys, pz)
                        ci += 1
                        i0 = ib_ * 1024 + hf * 512
                        nc.sync.dma_start(
                            out=yT[e8 * P:(e8 + 1) * P, i0:i0 + 512], in_=ys)

    nc.compile()
    return nc


_nc_by_reps = {}


def _get_nc(reps=1):
    if reps not in _nc_by_reps:
        _nc_by_reps[reps] = _build_nc(reps)
    return _nc_by_reps[reps]


def _make_in_maps(x_a, x_b, W_q, W_kv, W_out):
    import ml_dtypes
    bf = ml_dtypes.bfloat16
    f8 = ml_dtypes.float8_e4m3
    # fp8 weights pre-scaled x16 so W ~N(0, 0.32^2) sits in the e4m3
    # normal range (subnormals below 2^-6 would quantize catastrophically);
    # the resulting x256 on the logits is divided back out in the exp scale.
    xa8 = [np.ascontiguousarray(x_a[b].T).astype(f8) for b in range(B)]
    xb8 = [np.ascontiguousarray(x_b[b].T).astype(f8) for b in range(B)]
    xbT = [np.ascontiguousarray(x_b[b].T).astype(bf) for b in range(B)]
    ones = np.ones((P, P), dtype=bf)
    in_maps = []
    for c in range(NCORES):
        b, hh = divmod(c, 2)
        hs = hh * DVC
        in_maps.append({
            "xa8": xa8[b],
            "xb8": xb8[b],
            "xbT": xbT[b],
            "wq8": np.ascontiguousarray(
                (16.0 * W_q[hs:hs + DVC]).T).astype(f8),
            "wk8": np.ascontiguousarray(
                (16.0 * W_kv[hs:hs + DVC]).T).astype(f8),
            "wvT": np.ascontiguousarray(
                W_kv[DIM + hs:DIM + hs + DVC].T).astype(bf),
            "woT": np.ascontiguousarray(W_out[:, hs:hs + DVC].T).astype(bf),
            "ones": ones,
        })
    return in_maps


def kernel(x_a, x_b, W_q, W_kv, W_out, b_out):
    global LAST_EXEC_NS, LAST_RESULTS
    from concourse import bass_utils

    x_a = np.asarray(x_a, dtype=np.float32)
    x_b = np.asarray(x_b, dtype=np.float32)
    W_q = np.asarray(W_q, dtype=np.float32)
    W_kv = np.asarray(W_kv, dtype=np.float32)
    W_out = np.asarray(W_out, dtype=np.float32)
    b_out = np.asarray(b_out, dtype=np.float32)

    nc = _get_nc(REPS)
    in_maps = _make_in_maps(x_a, x_b, W_q, W_kv, W_out)

    res = bass_utils.run_bass_kernel_spmd(
        nc, in_maps, core_ids=list(range(NCORES)), trace=_TRACE,
        tmpdir=_TRACE_DIR)
    LAST_EXEC_NS = res.exec_time_ns
    LAST_RESULTS = res

    out = np.empty((B, N, DIM), dtype=np.float32)
    for b in range(B):
        acc = (np.asarray(res.results[2 * b]["yT"]).astype(np.float32)
               + np.asarray(res.results[2 * b + 1]["yT"]).astype(np.float32))
        out[b] = acc.T + b_out
    return out


def bench(inputs, reps_pair=(1, 9), iters=5):
    """Measure on-device time per kernel body via rep-delta wall timing."""
    import time
    from concourse import bass_utils
    ins = {k: np.asarray(v, dtype=np.float32) for k, v in inputs.items()
           if k != "b_out"}
    in_maps = _make_in_maps(ins["x_a"], ins["x_b"], ins["W_q"], ins["W_kv"],
                            ins["W_out"])
    walls = {}
    for reps in reps_pair:
        nc = _get_nc(reps)
        bass_utils.run_bass_kernel_spmd(nc, in_maps, core_ids=list(range(NCORES)))
        ts = []
        for _ in range(iters):
            t0 = time.perf_counter()
            bass_utils.run_bass_kernel_spmd(nc, in_maps,
                                            core_ids=list(range(NCORES)))
            ts.append(time.perf_counter() - t0)
        walls[reps] = min(ts)
        print(f"reps={reps}: wall min={walls[reps]*1e3:.2f} ms  "
              f"all={[f'{t*1e3:.1f}' for t in ts]}")
    r0, r1 = reps_pair
    ns = (walls[r1] - walls[r0]) / (r1 - r0) * 1e9
    print(f"per-body device time: {ns:.0f} ns")
    return ns
